# revision 47
# baseline (speedup 1.0000x reference)
"""Trainium2 Bass kernel for nn_DynamicAttention (trilinear attention).

Reference computation (per batch b; l=s=t=128, h=512):
    q     = query @ Wq + bq                  # [l, h]
    s_key = src @ Ws + bs                    # [s, h]
    t_key = trg @ Ws + bs                    # [t, h]
    w[l,s,t] = sum_k q[l,k] s_key[s,k] t_key[t,k] / sqrt(h)
    w     = softmax over the flattened (s,t) axis
    s_ctx = sum_{s,t} w * src[s,:] ; t_ctx = sum_{s,t} w * trg[t,:]
    out   = concat(query, s_ctx, t_ctx) @ Wo + bo

Sharding: data-parallel over batch. B=16 across 8 cores -> 2 batches/core;
no collectives. Inputs arrive host-pre-transposed/bf16-cast where useful.

Per-core device algorithm (everything stays on-chip; scores/E are never
spilled to HBM):
  - Projections as transposed bf16 matmuls producing qT[h,l], s_keyT[h,s],
    t_keyT[h,t] (weights natural as lhsT, h on partitions), biases applied
    on the PSUM->SBUF eviction (scalar-engine activation, per-partition).
  - Scores laid out [l, s*t] and processed in 16 chunks of 8 s-values
    (free width 1024). Per chunk the rank-1 factor matrix
    M[k, s*128+t] = s_keyT[k,s]*t_keyT[k,t] is materialized bf16:
    one plane per contraction block (kc) either built directly on the
    vector engine via broadcast APs (1x mode), or - to offload DVE - as a
    dense s_rep replica built on the scalar engine followed by a fully
    dense bf16 tensor_tensor multiply against a per-batch t_rep tile,
    which engages the DVE 2x packed mode.  Scores accumulate over 4 kc
    planes into PSUM via bf16 matmuls (lhsT = qT chunk, moving = M).
  - exp runs on the scalar engine straight out of PSUM (scale folded in);
    E is stored bf16.  Softmax marginals: ws[l,s]=sum_t E via per-chunk
    X-axis reduces (DVE), wt[l,t]=sum_s E via per-chunk accumulation on
    GPSIMD; Z = sum ws, both marginals scaled by 1/Z, then transposed on
    the PE (identity trick) to wsT[s,l]/wtT[t,l].
  - Contexts as single fp32 matmuls: s_ctxT[f,l] = src[s,f-chunk]^T-free
    matmul with rhs=wsT (src natural as lhsT), same for t_ctxT with trg.
  - Final: out[l,:] = sum_j X^T[j-chunk]^T @ Wo[j-chunk,:] + bo in fp32,
    with X^T = [queryT(f32, host-provided); s_ctxT; t_ctxT].

Numerics: the score chain runs in bf16 (inputs/weights/M/E); this is safe
here because the softmax is near-uniform (scores ~ +-0.4), so relative
weight errors stay ~1e-3 and the final output error is ~2e-5 relative.
The query passthrough, context matmuls, and output matmul stay fp32.
"""

import math

import numpy as np

import concourse.bass as bass
import concourse.bacc as bacc
import concourse.mybir as mybir
import concourse.tile as tile
from concourse.bass_utils import run_bass_kernel_spmd
from concourse.masks import make_identity

F32 = mybir.dt.float32
F32R = mybir.dt.float32r
BF16 = mybir.dt.bfloat16

B, L, S, T = 16, 128, 128, 128
HID, QDIM, FDIM, ODIM = 512, 512, 512, 512
NCORES = 8
BPC = B // NCORES  # batches per core
P = 128
KC = HID // P          # 4 contraction chunks
SBLK = 8               # s-values per score chunk
NCHUNK = S // SBLK     # 16 chunks per batch, each [128, SBLK*128=1024]
CW = SBLK * T          # chunk free width = 1024
JC = (QDIM + 2 * FDIM) // P  # 12 final-matmul contraction chunks

def _mbuild_route(j, kc):
    # "direct" = DVE broadcast-AP build (1x mode); "act" = scalar engine
    # materializes a dense s_rep, then DVE multiplies at 2x packed mode.
    # Chosen to balance DVE vs scalar-engine occupancy (measured).
    return ("direct", "act", "act", "act")[kc]
# wt-accumulation executor per chunk index: every WT_DVE_EVERYth chunk on
# DVE, the rest on GPSIMD.
WT_DVE_EVERY = 5


def _build_nc():
    nc = bacc.Bacc("TRN2", target_bir_lowering=False, debug=False)

    d_qT = nc.dram_tensor("qT", [QDIM, BPC * L], BF16, kind="ExternalInput")
    d_qTf = nc.dram_tensor("qTf", [QDIM, BPC * L], F32, kind="ExternalInput")
    d_sT = nc.dram_tensor("sT", [FDIM, BPC * S], BF16, kind="ExternalInput")
    d_tT = nc.dram_tensor("tT", [FDIM, BPC * T], BF16, kind="ExternalInput")
    d_src = nc.dram_tensor("src", [BPC * S, FDIM], F32, kind="ExternalInput")
    d_trg = nc.dram_tensor("trg", [BPC * T, FDIM], F32, kind="ExternalInput")
    d_Wq = nc.dram_tensor("Wq", [QDIM, HID], BF16, kind="ExternalInput")
    d_Ws = nc.dram_tensor("Ws", [FDIM, HID], BF16, kind="ExternalInput")
    d_Wo = nc.dram_tensor("Wo", [QDIM + 2 * FDIM, ODIM], F32, kind="ExternalInput")
    d_bq = nc.dram_tensor("bq", [HID], F32, kind="ExternalInput")
    d_bs = nc.dram_tensor("bs", [HID], F32, kind="ExternalInput")
    d_bo = nc.dram_tensor("bo", [ODIM], F32, kind="ExternalInput")
    d_out = nc.dram_tensor("out", [BPC * L, ODIM], F32, kind="ExternalOutput")

    N = BPC * L  # 256: both batches side by side in the free dim

    with tile.TileContext(nc) as tc:
        with (
            tc.tile_pool(name="const", bufs=1) as const,
            tc.tile_pool(name="acts", bufs=1) as acts,
            tc.tile_pool(name="epool", bufs=1) as epool,
            tc.tile_pool(name="mpool", bufs=5) as mpool,
            tc.tile_pool(name="srpool", bufs=3) as srpool,
            tc.tile_pool(name="trpool", bufs=1) as trpool,
            tc.tile_pool(name="small", bufs=2) as small,
            tc.tile_pool(name="ps_score", bufs=2, space="PSUM") as ps_score,
            tc.tile_pool(name="ps_misc", bufs=4, space="PSUM") as ps_misc,
        ):
            # ---- load constants / inputs ----
            qT_sb = const.tile([P, KC, N], BF16)
            sT_sb = const.tile([P, KC, N], BF16)
            tT_sb = const.tile([P, KC, N], BF16)
            qTf_sb = const.tile([P, KC, N], F32)
            nc.sync.dma_start(out=qT_sb, in_=d_qT.rearrange("(c p) n -> p c n", p=P))
            nc.sync.dma_start(out=sT_sb, in_=d_sT.rearrange("(c p) n -> p c n", p=P))
            nc.sync.dma_start(out=tT_sb, in_=d_tT.rearrange("(c p) n -> p c n", p=P))

            Wq_sb = const.tile([P, KC, HID], BF16)
            Ws_sb = const.tile([P, KC, HID], BF16)
            nc.sync.dma_start(out=Wq_sb, in_=d_Wq.rearrange("(c p) h -> p c h", p=P))
            nc.sync.dma_start(out=Ws_sb, in_=d_Ws.rearrange("(c p) h -> p c h", p=P))
            Wo_sb = const.tile([P, JC, ODIM], F32)

            bq_sb = const.tile([P, KC], F32)
            bs_sb = const.tile([P, KC], F32)
            nc.sync.dma_start(out=bq_sb, in_=d_bq.rearrange("(c p) -> p c", p=P))
            nc.sync.dma_start(out=bs_sb, in_=d_bs.rearrange("(c p) -> p c", p=P))
            bo_sb = const.tile([P, ODIM], F32)

            src_sb = const.tile([P, BPC, FDIM], F32)
            trg_sb = const.tile([P, BPC, FDIM], F32)

            ident = const.tile([P, P], F32)
            make_identity(nc, ident[:])

            # ---- projections: xT[h, n] = W^T @ inputT, + bias ----
            q_sb = acts.tile([P, KC, N], BF16)
            sk_sb = acts.tile([P, KC, N], BF16)
            tk_sb = acts.tile([P, KC, N], BF16)
            # batch 0's projections are emitted first so its score chunks
            # can start while batch 1's projections run in the background
            for pb in range(BPC):
                pbsl = slice(pb * P, (pb + 1) * P)
                for hc in range(KC):
                    for w_sb, x_sb, b_sb, o_sb in (
                        (Ws_sb, sT_sb, bs_sb, sk_sb),
                        (Ws_sb, tT_sb, bs_sb, tk_sb),
                        (Wq_sb, qT_sb, bq_sb, q_sb),
                    ):
                        pp = ps_misc.tile([P, P], F32, tag="misc")
                        for kc in range(KC):
                            nc.tensor.matmul(
                                pp[:],
                                w_sb[:, kc, hc * P : (hc + 1) * P],
                                x_sb[:, kc, pbsl],
                                start=(kc == 0),
                                stop=(kc == KC - 1),
                            )
                        nc.scalar.activation(
                            out=o_sb[:, hc, pbsl],
                            in_=pp[:],
                            func=mybir.ActivationFunctionType.Identity,
                            bias=b_sb[:, hc : hc + 1],
                            scale=1.0,
                        )

            # bulk tensors only needed from the batch tails onward -
            # deferred so the projection-critical DMAs go out first
            nc.sync.dma_start(out=qTf_sb, in_=d_qTf.rearrange("(c p) n -> p c n", p=P))
            nc.sync.dma_start(out=src_sb, in_=d_src.rearrange("(b s) f -> s b f", s=P))
            nc.sync.dma_start(out=trg_sb, in_=d_trg.rearrange("(b s) f -> s b f", s=P))
            nc.sync.dma_start(out=Wo_sb, in_=d_Wo.rearrange("(c p) o -> p c o", p=P))
            nc.sync.dma_start(
                out=bo_sb, in_=d_bo[:].unsqueeze(0).broadcast_to((P, ODIM))
            )

            ctxT_sb = acts.tile([P, 8, N], F32)
            inv_sqrt_h = 1.0 / math.sqrt(HID)

            for b in range(BPC):
                bsl = slice(b * P, (b + 1) * P)
                e_b = epool.tile([P, S, T], BF16, tag="e")
                ws = small.tile([P, S], F32, tag="ws")
                wt512 = small.tile([P, SBLK, T], F32, tag="wt512")
                wt = small.tile([P, T], F32, tag="wt")

                # t_rep[kc] = tk tiled SBLK times along the free dim (dense
                # bf16), built once per batch so the per-chunk M multiply is
                # a fully dense tensor_tensor (2x packed mode eligible).
                t_rep = trpool.tile([P, KC, SBLK, T], BF16, tag="trep")
                for kc in range(KC):
                    nc.scalar.activation(
                        out=t_rep[:, kc],
                        in_=tk_sb[:, kc, bsl].unsqueeze(1)
                        .broadcast_to((P, SBLK, T)),
                        func=mybir.ActivationFunctionType.Identity,
                        scale=1.0,
                    )

                op_ps = ps_misc.tile([P, ODIM], F32, tag="misc")
                for jc in range(KC):
                    nc.tensor.matmul(
                        op_ps[:], qTf_sb[:, jc, bsl], Wo_sb[:, jc, :],
                        start=(jc == 0), stop=False,
                    )

                for j in range(NCHUNK):
                    scol = slice(b * P + SBLK * j, b * P + SBLK * (j + 1))
                    s_rep = srpool.tile([P, KC, SBLK, T], BF16, tag="srep")
                    for kc in range(KC):
                        route = _mbuild_route(j, kc)
                        if route == "act":
                            nc.scalar.activation(
                                out=s_rep[:, kc],
                                in_=sk_sb[:, kc, scol].unsqueeze(2)
                                .broadcast_to((P, SBLK, T)),
                                func=mybir.ActivationFunctionType.Identity,
                                scale=1.0,
                            )
                        elif route == "gpsimd":
                            nc.gpsimd.tensor_copy(
                                out=s_rep[:, kc],
                                in_=sk_sb[:, kc, scol].unsqueeze(2)
                                .broadcast_to((P, SBLK, T)),
                            )
                    m_t = mpool.tile([P, KC, SBLK, T], BF16, tag="m")
                    for kc in range(KC):
                        if _mbuild_route(j, kc) != "direct":
                            nc.vector.tensor_tensor(
                                out=m_t[:, kc],
                                in0=t_rep[:, kc],
                                in1=s_rep[:, kc],
                                op=mybir.AluOpType.mult,
                            )
                        else:
                            nc.vector.tensor_tensor(
                                out=m_t[:, kc],
                                in0=tk_sb[:, kc, bsl].unsqueeze(1)
                                .broadcast_to((P, SBLK, T)),
                                in1=sk_sb[:, kc, scol].unsqueeze(2)
                                .broadcast_to((P, SBLK, T)),
                                op=mybir.AluOpType.mult,
                            )
                    sc_ps = ps_score.tile([P, CW], F32, tag="sc")
                    for h in range(CW // 512):
                        for kc in range(KC):
                            nc.tensor.matmul(
                                sc_ps[:, 512 * h : 512 * (h + 1)],
                                q_sb[:, kc, bsl],
                                m_t[:, kc]
                                .rearrange("p s t -> p (s t)")[
                                    :, 512 * h : 512 * (h + 1)
                                ],
                                start=(kc == 0),
                                stop=(kc == KC - 1),
                            )
                    e_chunk = e_b[:, SBLK * j : SBLK * (j + 1), :]
                    half = SBLK // 2
                    for h in range(2):
                        nc.scalar.activation(
                            out=e_b[
                                :, SBLK * j + half * h : SBLK * j + half * (h + 1), :
                            ].rearrange("p s t -> p (s t)"),
                            in_=sc_ps[:, CW // 2 * h : CW // 2 * (h + 1)],
                            func=mybir.ActivationFunctionType.Exp,
                            scale=inv_sqrt_h,
                        )
                    nc.vector.tensor_reduce(
                        out=ws[:, SBLK * j : SBLK * (j + 1)],
                        in_=e_chunk,
                        axis=mybir.AxisListType.X,
                        op=mybir.AluOpType.add,
                    )
                    # wt accumulation: wt512 += sum-over-chunk
                    wt_eng = nc.vector if j % WT_DVE_EVERY == 0 else nc.gpsimd
                    if j == 0:
                        wt_eng.tensor_copy(out=wt512[:], in_=e_chunk)
                    else:
                        wt_eng.tensor_tensor(
                            out=wt512[:], in0=wt512[:], in1=e_chunk,
                            op=mybir.AluOpType.add,
                        )

                # fold wt512 [P,SBLK,T] -> wt [P,T] (binary tree)
                width = SBLK
                while width > 2:
                    half = width // 2
                    nc.gpsimd.tensor_tensor(
                        out=wt512[:, :half],
                        in0=wt512[:, :half],
                        in1=wt512[:, half:width],
                        op=mybir.AluOpType.add,
                    )
                    width = half
                nc.gpsimd.tensor_tensor(
                    out=wt[:], in0=wt512[:, 0], in1=wt512[:, 1],
                    op=mybir.AluOpType.add,
                )

                # softmax denominator and marginals
                z = small.tile([P, 1], F32, tag="z")
                nc.vector.tensor_reduce(
                    out=z[:], in_=ws[:], axis=mybir.AxisListType.X,
                    op=mybir.AluOpType.add,
                )
                invz = small.tile([P, 1], F32, tag="invz")
                nc.vector.reciprocal(out=invz[:], in_=z[:])
                nc.vector.tensor_scalar_mul(ws[:], ws[:], invz[:])
                nc.vector.tensor_scalar_mul(wt[:], wt[:], invz[:])

                # transpose marginals: wsT[s, l], wtT[t, l]
                wsT = small.tile([P, P], F32, tag="wsT")
                wtT = small.tile([P, P], F32, tag="wtT")
                for w_in, w_out in ((ws, wsT), (wt, wtT)):
                    tp = ps_misc.tile([P, P], F32, tag="misc")
                    nc.tensor.transpose(tp[:], w_in[:], ident[:])
                    nc.vector.tensor_copy(out=w_out[:], in_=tp[:])

                # contexts: s_ctxT[f,l] = sum_s src[s,f]*wsT[s,l]
                for w_t, x_sb, off in ((wsT, src_sb, 0), (wtT, trg_sb, 4)):
                    for fc in range(KC):
                        cp = ps_misc.tile([P, P], F32, tag="misc")
                        nc.tensor.matmul(
                            cp[:],
                            x_sb[:, b, fc * P : (fc + 1) * P],
                            w_t[:],
                            start=True,
                            stop=True,
                        )
                        nc.scalar.activation(
                            out=ctxT_sb[:, off + fc, bsl],
                            in_=cp[:],
                            func=mybir.ActivationFunctionType.Identity,
                            scale=1.0,
                        )

                # final: remaining context planes accumulate onto the
                # query planes issued before the chunk loop
                for jc in range(KC, JC):
                    nc.tensor.matmul(
                        op_ps[:], ctxT_sb[:, jc - KC, bsl], Wo_sb[:, jc, :],
                        start=False, stop=(jc == JC - 1),
                    )
                out_sb = small.tile([P, ODIM], F32, tag="out")
                nc.vector.tensor_tensor(
                    out=out_sb[:], in0=op_ps[:], in1=bo_sb[:],
                    op=mybir.AluOpType.add,
                )
                nc.sync.dma_start(out=d_out[bsl, :], in_=out_sb[:])

    nc.compile()
    return nc


_NC_CACHE = None


def _get_nc():
    global _NC_CACHE
    if _NC_CACHE is None:
        _NC_CACHE = _build_nc()
    return _NC_CACHE




def _core_in_map(tensors, c):
    import ml_dtypes

    bf = ml_dtypes.bfloat16
    sl = slice(BPC * c, BPC * (c + 1))
    qs = np.asarray(tensors["query"], np.float32)[sl].reshape(BPC * L, QDIM)
    ss = np.asarray(tensors["src"], np.float32)[sl].reshape(BPC * S, FDIM)
    ts = np.asarray(tensors["trg"], np.float32)[sl].reshape(BPC * T, FDIM)
    return {
        "qT": np.ascontiguousarray(qs.T).astype(bf),
        "qTf": np.ascontiguousarray(qs.T),
        "sT": np.ascontiguousarray(ss.T).astype(bf),
        "tT": np.ascontiguousarray(ts.T).astype(bf),
        "src": np.ascontiguousarray(ss),
        "trg": np.ascontiguousarray(ts),
        "Wq": np.asarray(tensors["Wq"], np.float32).astype(bf),
        "Ws": np.asarray(tensors["Ws"], np.float32).astype(bf),
        "Wo": np.ascontiguousarray(np.asarray(tensors["Wo"], np.float32)),
        "bq": np.ascontiguousarray(np.asarray(tensors["bq"], np.float32)),
        "bs": np.ascontiguousarray(np.asarray(tensors["bs"], np.float32)),
        "bo": np.ascontiguousarray(np.asarray(tensors["bo"], np.float32)),
    }

def kernel(query, src, trg, Wq, bq, Ws, bs, Wo, bo):
    query = np.asarray(query, dtype=np.float32)
    src = np.asarray(src, dtype=np.float32)
    trg = np.asarray(trg, dtype=np.float32)
    Wq = np.ascontiguousarray(np.asarray(Wq, dtype=np.float32))
    Ws = np.ascontiguousarray(np.asarray(Ws, dtype=np.float32))
    Wo = np.ascontiguousarray(np.asarray(Wo, dtype=np.float32))
    bq = np.ascontiguousarray(np.asarray(bq, dtype=np.float32))
    bs = np.ascontiguousarray(np.asarray(bs, dtype=np.float32))
    bo = np.ascontiguousarray(np.asarray(bo, dtype=np.float32))

    nc = _get_nc()
    tensors = {
        "query": query, "src": src, "trg": trg,
        "Wq": Wq, "Ws": Ws, "Wo": Wo, "bq": bq, "bs": bs, "bo": bo,
    }
    in_maps = [_core_in_map(tensors, c) for c in range(NCORES)]
    global _last_in_maps
    _last_in_maps = in_maps
    res = run_bass_kernel_spmd(nc, in_maps, list(range(NCORES))).results
    out = np.concatenate(
        [res[c]["out"].reshape(BPC, L, ODIM) for c in range(NCORES)], axis=0
    )
    return out.astype(np.float32)


# revision 48
# speedup vs baseline: 1.0463x; 1.0463x over previous
"""Trainium2 Bass kernel for nn_DynamicAttention (trilinear attention).

Reference computation (per batch b; l=s=t=128, h=512):
    q     = query @ Wq + bq                  # [l, h]
    s_key = src @ Ws + bs                    # [s, h]
    t_key = trg @ Ws + bs                    # [t, h]
    w[l,s,t] = sum_k q[l,k] s_key[s,k] t_key[t,k] / sqrt(h)
    w     = softmax over the flattened (s,t) axis
    s_ctx = sum_{s,t} w * src[s,:] ; t_ctx = sum_{s,t} w * trg[t,:]
    out   = concat(query, s_ctx, t_ctx) @ Wo + bo

Sharding: data-parallel over batch. B=16 across 8 cores -> 2 batches/core;
no collectives. Inputs arrive host-pre-transposed/bf16-cast where useful.

Per-core device algorithm (everything stays on-chip; scores/E are never
spilled to HBM):
  - Projections as transposed bf16 matmuls producing qT[h,l], s_keyT[h,s],
    t_keyT[h,t] (weights natural as lhsT, h on partitions), biases applied
    on the PSUM->SBUF eviction (scalar-engine activation, per-partition).
  - Scores laid out [l, s*t] and processed in 16 chunks of 8 s-values
    (free width 1024). Per chunk the rank-1 factor matrix
    M[k, s*128+t] = s_keyT[k,s]*t_keyT[k,t] is materialized bf16:
    one plane per contraction block (kc) either built directly on the
    vector engine via broadcast APs (1x mode), or - to offload DVE - as a
    dense s_rep replica built on the scalar engine followed by a fully
    dense bf16 tensor_tensor multiply against a per-batch t_rep tile,
    which engages the DVE 2x packed mode.  Scores accumulate over 4 kc
    planes into PSUM via bf16 matmuls (lhsT = qT chunk, moving = M).
  - exp runs on the scalar engine straight out of PSUM (scale folded in);
    E is stored bf16.  Softmax marginals: ws[l,s]=sum_t E via per-chunk
    X-axis reduces (DVE), wt[l,t]=sum_s E via per-chunk accumulation on
    GPSIMD; Z = sum ws, both marginals scaled by 1/Z, then transposed on
    the PE (identity trick) to wsT[s,l]/wtT[t,l].
  - Contexts as single fp32 matmuls: s_ctxT[f,l] = src[s,f-chunk]^T-free
    matmul with rhs=wsT (src natural as lhsT), same for t_ctxT with trg.
  - Final: out[l,:] = sum_j X^T[j-chunk]^T @ Wo[j-chunk,:] + bo in fp32,
    with X^T = [queryT(f32, host-provided); s_ctxT; t_ctxT].

Numerics: the score chain runs in bf16 (inputs/weights/M/E); this is safe
here because the softmax is near-uniform (scores ~ +-0.4), so relative
weight errors stay ~1e-3 and the final output error is ~2e-5 relative.
The query passthrough, context matmuls, and output matmul stay fp32.
"""

import math

import numpy as np

import concourse.bass as bass
import concourse.bacc as bacc
import concourse.mybir as mybir
import concourse.tile as tile
from concourse.bass_utils import run_bass_kernel_spmd
from concourse.masks import make_identity

F32 = mybir.dt.float32
F32R = mybir.dt.float32r
BF16 = mybir.dt.bfloat16

B, L, S, T = 16, 128, 128, 128
HID, QDIM, FDIM, ODIM = 512, 512, 512, 512
NCORES = 8
BPC = B // NCORES  # batches per core
P = 128
KC = HID // P          # 4 contraction chunks
SBLK = 8               # s-values per score chunk
NCHUNK = S // SBLK     # 16 chunks per batch, each [128, SBLK*128=1024]
CW = SBLK * T          # chunk free width = 1024
JC = (QDIM + 2 * FDIM) // P  # 12 final-matmul contraction chunks

def _mbuild_route(j, kc):
    # "direct" = DVE broadcast-AP build (1x mode); "act" = scalar engine
    # materializes a dense s_rep, then DVE multiplies at 2x packed mode.
    # Chosen to balance DVE vs scalar-engine occupancy (measured).
    return ("direct", "act", "act", "act" if j % 2 == 0 else "direct")[kc]
# wt-accumulation executor per chunk index: every WT_DVE_EVERYth chunk on
# DVE, the rest on GPSIMD.
WT_DVE_EVERY = 5


def _build_nc():
    nc = bacc.Bacc("TRN2", target_bir_lowering=False, debug=False)

    d_qT = nc.dram_tensor("qT", [QDIM, BPC * L], BF16, kind="ExternalInput")
    d_qTf = nc.dram_tensor("qTf", [QDIM, BPC * L], F32, kind="ExternalInput")
    d_sT = nc.dram_tensor("sT", [FDIM, BPC * S], BF16, kind="ExternalInput")
    d_tT = nc.dram_tensor("tT", [FDIM, BPC * T], BF16, kind="ExternalInput")
    d_src = nc.dram_tensor("src", [BPC * S, FDIM], F32, kind="ExternalInput")
    d_trg = nc.dram_tensor("trg", [BPC * T, FDIM], F32, kind="ExternalInput")
    d_Wq = nc.dram_tensor("Wq", [QDIM, HID], BF16, kind="ExternalInput")
    d_Ws = nc.dram_tensor("Ws", [FDIM, HID], BF16, kind="ExternalInput")
    d_Wo = nc.dram_tensor("Wo", [QDIM + 2 * FDIM, ODIM], F32, kind="ExternalInput")
    d_bq = nc.dram_tensor("bq", [HID], F32, kind="ExternalInput")
    d_bs = nc.dram_tensor("bs", [HID], F32, kind="ExternalInput")
    d_bo = nc.dram_tensor("bo", [ODIM], F32, kind="ExternalInput")
    d_out = nc.dram_tensor("out", [BPC * L, ODIM], F32, kind="ExternalOutput")

    N = BPC * L  # 256: both batches side by side in the free dim

    with tile.TileContext(nc) as tc:
        with (
            tc.tile_pool(name="const", bufs=1) as const,
            tc.tile_pool(name="acts", bufs=1) as acts,
            tc.tile_pool(name="epool", bufs=1) as epool,
            tc.tile_pool(name="mpool", bufs=5) as mpool,
            tc.tile_pool(name="srpool", bufs=3) as srpool,
            tc.tile_pool(name="trpool", bufs=1) as trpool,
            tc.tile_pool(name="small", bufs=2) as small,
            tc.tile_pool(name="ps_score", bufs=2, space="PSUM") as ps_score,
            tc.tile_pool(name="ps_misc", bufs=4, space="PSUM") as ps_misc,
        ):
            # ---- load constants / inputs ----
            qT_sb = const.tile([P, KC, N], BF16)
            sT_sb = const.tile([P, KC, N], BF16)
            tT_sb = const.tile([P, KC, N], BF16)
            qTf_sb = const.tile([P, KC, N], F32)
            nc.sync.dma_start(out=qT_sb, in_=d_qT.rearrange("(c p) n -> p c n", p=P))
            nc.sync.dma_start(out=sT_sb, in_=d_sT.rearrange("(c p) n -> p c n", p=P))
            nc.sync.dma_start(out=tT_sb, in_=d_tT.rearrange("(c p) n -> p c n", p=P))

            Wq_sb = const.tile([P, KC, HID], BF16)
            Ws_sb = const.tile([P, KC, HID], BF16)
            nc.sync.dma_start(out=Wq_sb, in_=d_Wq.rearrange("(c p) h -> p c h", p=P))
            nc.sync.dma_start(out=Ws_sb, in_=d_Ws.rearrange("(c p) h -> p c h", p=P))
            Wo_sb = const.tile([P, JC, ODIM], F32)

            bq_sb = const.tile([P, KC], F32)
            bs_sb = const.tile([P, KC], F32)
            nc.sync.dma_start(out=bq_sb, in_=d_bq.rearrange("(c p) -> p c", p=P))
            nc.sync.dma_start(out=bs_sb, in_=d_bs.rearrange("(c p) -> p c", p=P))
            bo_sb = const.tile([P, ODIM], F32)

            src_sb = const.tile([P, BPC, FDIM], F32)
            trg_sb = const.tile([P, BPC, FDIM], F32)

            ident = const.tile([P, P], F32)
            make_identity(nc, ident[:])

            # ---- projections: xT[h, n] = W^T @ inputT, + bias ----
            q_sb = acts.tile([P, KC, N], BF16)
            sk_sb = acts.tile([P, KC, N], BF16)
            tk_sb = acts.tile([P, KC, N], BF16)
            # batch 0's projections are emitted first so its score chunks
            # can start while batch 1's projections run in the background
            for pb in range(BPC):
                pbsl = slice(pb * P, (pb + 1) * P)
                for hc in range(KC):
                    for w_sb, x_sb, b_sb, o_sb in (
                        (Ws_sb, sT_sb, bs_sb, sk_sb),
                        (Ws_sb, tT_sb, bs_sb, tk_sb),
                        (Wq_sb, qT_sb, bq_sb, q_sb),
                    ):
                        pp = ps_misc.tile([P, P], F32, tag="misc")
                        for kc in range(KC):
                            nc.tensor.matmul(
                                pp[:],
                                w_sb[:, kc, hc * P : (hc + 1) * P],
                                x_sb[:, kc, pbsl],
                                start=(kc == 0),
                                stop=(kc == KC - 1),
                            )
                        nc.scalar.activation(
                            out=o_sb[:, hc, pbsl],
                            in_=pp[:],
                            func=mybir.ActivationFunctionType.Identity,
                            bias=b_sb[:, hc : hc + 1],
                            scale=1.0,
                        )

            # bulk tensors only needed from the batch tails onward -
            # deferred so the projection-critical DMAs go out first
            nc.sync.dma_start(out=qTf_sb, in_=d_qTf.rearrange("(c p) n -> p c n", p=P))
            nc.sync.dma_start(out=src_sb, in_=d_src.rearrange("(b s) f -> s b f", s=P))
            nc.sync.dma_start(out=trg_sb, in_=d_trg.rearrange("(b s) f -> s b f", s=P))
            nc.sync.dma_start(out=Wo_sb, in_=d_Wo.rearrange("(c p) o -> p c o", p=P))
            nc.sync.dma_start(
                out=bo_sb, in_=d_bo[:].unsqueeze(0).broadcast_to((P, ODIM))
            )

            ctxT_sb = acts.tile([P, 8, N], F32)
            inv_sqrt_h = 1.0 / math.sqrt(HID)

            for b in range(BPC):
                bsl = slice(b * P, (b + 1) * P)
                e_b = epool.tile([P, S, T], BF16, tag="e")
                ws = small.tile([P, S], F32, tag="ws")
                wt512 = small.tile([P, SBLK, T], F32, tag="wt512")
                wt = small.tile([P, T], F32, tag="wt")

                # t_rep[kc] = tk tiled SBLK times along the free dim (dense
                # bf16), built once per batch so the per-chunk M multiply is
                # a fully dense tensor_tensor (2x packed mode eligible).
                t_rep = trpool.tile([P, KC, SBLK, T], BF16, tag="trep")
                for kc in range(KC):
                    nc.scalar.activation(
                        out=t_rep[:, kc],
                        in_=tk_sb[:, kc, bsl].unsqueeze(1)
                        .broadcast_to((P, SBLK, T)),
                        func=mybir.ActivationFunctionType.Identity,
                        scale=1.0,
                    )

                op_ps = ps_misc.tile([P, ODIM], F32, tag="misc")
                for jc in range(KC):
                    nc.tensor.matmul(
                        op_ps[:], qTf_sb[:, jc, bsl], Wo_sb[:, jc, :],
                        start=(jc == 0), stop=False,
                    )

                for j in range(NCHUNK):
                    scol = slice(b * P + SBLK * j, b * P + SBLK * (j + 1))
                    s_rep = srpool.tile([P, KC, SBLK, T], BF16, tag="srep")
                    for kc in range(KC):
                        route = _mbuild_route(j, kc)
                        if route == "act":
                            nc.scalar.activation(
                                out=s_rep[:, kc],
                                in_=sk_sb[:, kc, scol].unsqueeze(2)
                                .broadcast_to((P, SBLK, T)),
                                func=mybir.ActivationFunctionType.Identity,
                                scale=1.0,
                            )
                        elif route == "gpsimd":
                            nc.gpsimd.tensor_copy(
                                out=s_rep[:, kc],
                                in_=sk_sb[:, kc, scol].unsqueeze(2)
                                .broadcast_to((P, SBLK, T)),
                            )
                    m_t = mpool.tile([P, KC, SBLK, T], BF16, tag="m")
                    for kc in range(KC):
                        if _mbuild_route(j, kc) != "direct":
                            nc.vector.tensor_tensor(
                                out=m_t[:, kc],
                                in0=t_rep[:, kc],
                                in1=s_rep[:, kc],
                                op=mybir.AluOpType.mult,
                            )
                        else:
                            nc.vector.tensor_tensor(
                                out=m_t[:, kc],
                                in0=tk_sb[:, kc, bsl].unsqueeze(1)
                                .broadcast_to((P, SBLK, T)),
                                in1=sk_sb[:, kc, scol].unsqueeze(2)
                                .broadcast_to((P, SBLK, T)),
                                op=mybir.AluOpType.mult,
                            )
                    sc_ps = ps_score.tile([P, CW], F32, tag="sc")
                    for h in range(CW // 512):
                        for kc in range(KC):
                            nc.tensor.matmul(
                                sc_ps[:, 512 * h : 512 * (h + 1)],
                                q_sb[:, kc, bsl],
                                m_t[:, kc]
                                .rearrange("p s t -> p (s t)")[
                                    :, 512 * h : 512 * (h + 1)
                                ],
                                start=(kc == 0),
                                stop=(kc == KC - 1),
                            )
                    e_chunk = e_b[:, SBLK * j : SBLK * (j + 1), :]
                    half = SBLK // 2
                    for h in range(2):
                        nc.scalar.activation(
                            out=e_b[
                                :, SBLK * j + half * h : SBLK * j + half * (h + 1), :
                            ].rearrange("p s t -> p (s t)"),
                            in_=sc_ps[:, CW // 2 * h : CW // 2 * (h + 1)],
                            func=mybir.ActivationFunctionType.Exp,
                            scale=inv_sqrt_h,
                        )
                    nc.vector.tensor_reduce(
                        out=ws[:, SBLK * j : SBLK * (j + 1)],
                        in_=e_chunk,
                        axis=mybir.AxisListType.X,
                        op=mybir.AluOpType.add,
                    )
                    # wt accumulation: wt512 += sum-over-chunk
                    wt_eng = nc.vector if j % WT_DVE_EVERY == 0 else nc.gpsimd
                    if j == 0:
                        wt_eng.tensor_copy(out=wt512[:], in_=e_chunk)
                    else:
                        wt_eng.tensor_tensor(
                            out=wt512[:], in0=wt512[:], in1=e_chunk,
                            op=mybir.AluOpType.add,
                        )

                # fold wt512 [P,SBLK,T] -> wt [P,T] (binary tree)
                width = SBLK
                while width > 2:
                    half = width // 2
                    nc.gpsimd.tensor_tensor(
                        out=wt512[:, :half],
                        in0=wt512[:, :half],
                        in1=wt512[:, half:width],
                        op=mybir.AluOpType.add,
                    )
                    width = half
                nc.gpsimd.tensor_tensor(
                    out=wt[:], in0=wt512[:, 0], in1=wt512[:, 1],
                    op=mybir.AluOpType.add,
                )

                # softmax denominator and marginals
                z = small.tile([P, 1], F32, tag="z")
                nc.vector.tensor_reduce(
                    out=z[:], in_=ws[:], axis=mybir.AxisListType.X,
                    op=mybir.AluOpType.add,
                )
                invz = small.tile([P, 1], F32, tag="invz")
                nc.vector.reciprocal(out=invz[:], in_=z[:])
                nc.vector.tensor_scalar_mul(ws[:], ws[:], invz[:])
                nc.vector.tensor_scalar_mul(wt[:], wt[:], invz[:])

                # transpose marginals: wsT[s, l], wtT[t, l]
                wsT = small.tile([P, P], F32, tag="wsT")
                wtT = small.tile([P, P], F32, tag="wtT")
                for w_in, w_out in ((ws, wsT), (wt, wtT)):
                    tp = ps_misc.tile([P, P], F32, tag="misc")
                    nc.tensor.transpose(tp[:], w_in[:], ident[:])
                    nc.vector.tensor_copy(out=w_out[:], in_=tp[:])

                # contexts: s_ctxT[f,l] = sum_s src[s,f]*wsT[s,l]
                for w_t, x_sb, off in ((wsT, src_sb, 0), (wtT, trg_sb, 4)):
                    for fc in range(KC):
                        cp = ps_misc.tile([P, P], F32, tag="misc")
                        nc.tensor.matmul(
                            cp[:],
                            x_sb[:, b, fc * P : (fc + 1) * P],
                            w_t[:],
                            start=True,
                            stop=True,
                        )
                        nc.scalar.activation(
                            out=ctxT_sb[:, off + fc, bsl],
                            in_=cp[:],
                            func=mybir.ActivationFunctionType.Identity,
                            scale=1.0,
                        )

                # final: remaining context planes accumulate onto the
                # query planes issued before the chunk loop
                for jc in range(KC, JC):
                    nc.tensor.matmul(
                        op_ps[:], ctxT_sb[:, jc - KC, bsl], Wo_sb[:, jc, :],
                        start=False, stop=(jc == JC - 1),
                    )
                out_sb = small.tile([P, ODIM], F32, tag="out")
                nc.vector.tensor_tensor(
                    out=out_sb[:], in0=op_ps[:], in1=bo_sb[:],
                    op=mybir.AluOpType.add,
                )
                nc.sync.dma_start(out=d_out[bsl, :], in_=out_sb[:])

    nc.compile()
    return nc


_NC_CACHE = None


def _get_nc():
    global _NC_CACHE
    if _NC_CACHE is None:
        _NC_CACHE = _build_nc()
    return _NC_CACHE




def _core_in_map(tensors, c):
    import ml_dtypes

    bf = ml_dtypes.bfloat16
    sl = slice(BPC * c, BPC * (c + 1))
    qs = np.asarray(tensors["query"], np.float32)[sl].reshape(BPC * L, QDIM)
    ss = np.asarray(tensors["src"], np.float32)[sl].reshape(BPC * S, FDIM)
    ts = np.asarray(tensors["trg"], np.float32)[sl].reshape(BPC * T, FDIM)
    return {
        "qT": np.ascontiguousarray(qs.T).astype(bf),
        "qTf": np.ascontiguousarray(qs.T),
        "sT": np.ascontiguousarray(ss.T).astype(bf),
        "tT": np.ascontiguousarray(ts.T).astype(bf),
        "src": np.ascontiguousarray(ss),
        "trg": np.ascontiguousarray(ts),
        "Wq": np.asarray(tensors["Wq"], np.float32).astype(bf),
        "Ws": np.asarray(tensors["Ws"], np.float32).astype(bf),
        "Wo": np.ascontiguousarray(np.asarray(tensors["Wo"], np.float32)),
        "bq": np.ascontiguousarray(np.asarray(tensors["bq"], np.float32)),
        "bs": np.ascontiguousarray(np.asarray(tensors["bs"], np.float32)),
        "bo": np.ascontiguousarray(np.asarray(tensors["bo"], np.float32)),
    }

def kernel(query, src, trg, Wq, bq, Ws, bs, Wo, bo):
    query = np.asarray(query, dtype=np.float32)
    src = np.asarray(src, dtype=np.float32)
    trg = np.asarray(trg, dtype=np.float32)
    Wq = np.ascontiguousarray(np.asarray(Wq, dtype=np.float32))
    Ws = np.ascontiguousarray(np.asarray(Ws, dtype=np.float32))
    Wo = np.ascontiguousarray(np.asarray(Wo, dtype=np.float32))
    bq = np.ascontiguousarray(np.asarray(bq, dtype=np.float32))
    bs = np.ascontiguousarray(np.asarray(bs, dtype=np.float32))
    bo = np.ascontiguousarray(np.asarray(bo, dtype=np.float32))

    nc = _get_nc()
    tensors = {
        "query": query, "src": src, "trg": trg,
        "Wq": Wq, "Ws": Ws, "Wo": Wo, "bq": bq, "bs": bs, "bo": bo,
    }
    in_maps = [_core_in_map(tensors, c) for c in range(NCORES)]
    global _last_in_maps
    _last_in_maps = in_maps
    res = run_bass_kernel_spmd(nc, in_maps, list(range(NCORES))).results
    out = np.concatenate(
        [res[c]["out"].reshape(BPC, L, ODIM) for c in range(NCORES)], axis=0
    )
    return out.astype(np.float32)
